# revision 5
# baseline (speedup 1.0000x reference)
# Trainium2 Bass kernel for nn_CrossAttention (B=2, Nq=4096, Nk=2048, D=128,
# Dv=768, H=4, hd=32).
#
# Sharding: data-parallel over (B x Nq-blocks): core c handles batch c//4,
# query rows (c%4)*1024 .. +1024. K/V/weights replicated per core.
#
# Math (host-folded):
#   qn = (q * rstd_q) @ WqT_eff + bq/sqrt(hd)   with WqT_eff = diag(rms_q_w) Wq^T / sqrt(hd)
#   kn = (k * rstd_k) @ WkT_eff + bk            with WkT_eff = diag(rms_k_w) Wk^T
#   S_h = qn_h kn_h^T  (scale already folded into q side)
#   A   = sum_h exp(S_h) / rowsum_h(exp S_h)    (no max subtraction: |S| < 8)
#   out = A @ (0.25 * V)
import numpy as np

B, NQ, NK, D, DV = 2, 4096, 2048, 128, 768
H, HD = 4, 32
N_CORES = 8
NQC = NQ * B // N_CORES  # 1024 queries per core
NQT = NQC // 128  # 8 query tiles per core
NKT = NK // 128  # 16 key tiles
RMS_EPS = 1.1920929e-07

_CACHE = {}


def _build_nc():
    import concourse.bacc as bacc
    import concourse.mybir as mybir
    import concourse.tile as tile

    fp32 = mybir.dt.float32
    f16 = mybir.dt.float16

    nc = bacc.Bacc("TRN2", target_bir_lowering=False, debug=False)

    q_d = nc.dram_tensor("q", [NQC, D], f16, kind="ExternalInput").ap()
    k_d = nc.dram_tensor("k", [NK, D], f16, kind="ExternalInput").ap()
    v_d = nc.dram_tensor("v", [NK, DV], f16, kind="ExternalInput").ap()
    wq_d = nc.dram_tensor("wqt", [D, D], f16, kind="ExternalInput").ap()
    wk_d = nc.dram_tensor("wkt", [D, D], f16, kind="ExternalInput").ap()
    bq_d = nc.dram_tensor("bqe", [D], fp32, kind="ExternalInput").ap()
    bk_d = nc.dram_tensor("bke", [D], fp32, kind="ExternalInput").ap()
    o_d = nc.dram_tensor("o", [NQC, DV], fp32, kind="ExternalOutput").ap()

    with tile.TileContext(nc) as tc:
        _tile_kernel(tc, o_d, q_d, k_d, v_d, wq_d, wk_d, bq_d, bk_d)
    nc.compile()
    return nc


def _tile_kernel(tc, o_d, q_d, k_d, v_d, wq_d, wk_d, bq_d, bk_d):
    from contextlib import ExitStack

    import concourse.mybir as mybir

    nc = tc.nc
    fp32 = mybir.dt.float32
    f16 = mybir.dt.float16
    AF = mybir.ActivationFunctionType
    OP = mybir.AluOpType
    AX = mybir.AxisListType

    ctx = ExitStack()
    with ctx:
        singles = ctx.enter_context(tc.tile_pool(name="singles", bufs=1))

        # junk: zero-filled source for PE warm-up matmuls (memset on the
        # otherwise-idle Pool queue so nothing else is delayed).
        junk = singles.tile([128, 512], f16)
        nc.gpsimd.memset(junk, 0.0)

        # --- input loads. kx/qx on the Sync dispatcher (transposes follow
        # there); weights/bias/v on the idle GpSimd dispatcher so the Scalar
        # queue is free for the stats Sqrts from t=0.
        kx_sb = singles.tile([128, NKT, D], f16)
        nc.sync.dma_start(out=kx_sb, in_=k_d.rearrange("(p c) d -> p c d", c=NKT))
        qx_sb = singles.tile([128, NQT, D], f16)
        nc.sync.dma_start(out=qx_sb, in_=q_d.rearrange("(p c) d -> p c d", c=NQT))
        wq_sb = singles.tile([128, D], f16)
        nc.gpsimd.dma_start(out=wq_sb, in_=wq_d)
        wk_sb = singles.tile([128, D], f16)
        nc.gpsimd.dma_start(out=wk_sb, in_=wk_d)
        bq_sb = singles.tile([128, 1], fp32)
        nc.gpsimd.dma_start(out=bq_sb, in_=bq_d[:, None])
        bk_sb = singles.tile([128, 1], fp32)
        nc.gpsimd.dma_start(out=bk_sb, in_=bk_d[:, None])
        # v queued behind everything else; only PV needs it (~20us in)
        v_sb = singles.tile([128, NKT, DV], f16)
        nc.gpsimd.dma_start(out=v_sb, in_=v_d.rearrange("(p c) d -> p c d", c=NKT))

        eps_sb = singles.tile([128, 1], fp32)
        nc.vector.memset(eps_sb, RMS_EPS)

        kxT = singles.tile([128, NK], f16)  # normalized, transposed [d, tok]
        qxT = singles.tile([128, NQC], f16)
        kT = singles.tile([128, NK], f16)  # projected (head h rows 32h..32h+31)
        qT = singles.tile([128, NQC], f16)

        # ---- preamble: RMSNorm + transpose + projections pipelined at
        # 512-token granularity (4-tile stats chunks). PE warm-up on junk
        # from t=0 keeps the HAM utilization window full so the 1.2->2.4 GHz
        # un-throttle fires as early as possible.
        pre_ctx = ExitStack()
        pre = pre_ctx.enter_context(tc.tile_pool(name="pre", bufs=1))
        prepsum = pre_ctx.enter_context(
            tc.tile_pool(name="prepsum", bufs=2, space="PSUM")
        )

        warm = prepsum.tile([128, 512], fp32, tag="warm", bufs=1)
        for _ in range(8):
            nc.tensor.matmul(
                warm, lhsT=junk[:, 0:128], rhs=junk, start=True, stop=True
            )

        def mk_side(nt, tag):
            ssq = pre.tile([128, nt], fp32, tag=f"ssq{tag}", name=f"ssq{tag}")
            sd = pre.tile([128, nt], fp32, tag=f"sd{tag}", name=f"sd{tag}")
            rstd = pre.tile([128, nt], fp32, tag=f"rstd{tag}", name=f"rstd{tag}")
            xn = pre.tile([128, nt, D], f16, tag=f"xn{tag}", name=f"xn{tag}")
            return ssq, sd, rstd, xn

        def stats_chunk(x_sb, side, ci, sfx):
            # 4 consecutive p-outer tiles: tokens for xT columns ci*512..+512
            ssq, sd, rstd, xn = side
            sl = slice(ci * 4, (ci + 1) * 4)
            sq = pre.tile([128, 4, D], f16, tag="sqh", bufs=2, name=f"sq_{sfx}")
            nc.vector.tensor_mul(sq, x_sb[:, sl, :], x_sb[:, sl, :])
            nc.vector.tensor_reduce(ssq[:, sl, None], sq, AX.X, OP.add)
            nc.scalar.activation(
                sd[:, sl], ssq[:, sl], AF.Sqrt, bias=eps_sb, scale=1.0 / D
            )
            nc.vector.reciprocal(rstd[:, sl], sd[:, sl])
            for t in range(ci * 4, (ci + 1) * 4):
                nc.vector.tensor_scalar_mul(
                    xn[:, t, :], x_sb[:, t, :], rstd[:, t : t + 1]
                )

        def transpose_chunk(xn, xT, ci):
            nc.sync.dma_start_transpose(
                out=xT[:, ci * 512 : (ci + 1) * 512].rearrange(
                    "p (c j) -> p c j", j=128
                ),
                in_=xn[:, ci * 4 : (ci + 1) * 4, :].rearrange("p c j -> p (c j)"),
            )

        _pj = [0]

        def proj(xT, w_sb, b_sb, dst, j, psum_pool=None):
            # eviction on DVE (tensor_scalar add) to keep ACT free for exp
            _pj[0] += 1
            pool = psum_pool if psum_pool is not None else prepsum
            if psum_pool is None:
                pp = pool.tile([128, 512], fp32, tag="proj", name=f"pp{_pj[0]}")
            else:
                pp = pool.tile([128, 1024], fp32, tag="S", name=f"pp{_pj[0]}")[
                    :, 0:512
                ]
            nc.tensor.matmul(
                pp, lhsT=w_sb, rhs=xT[:, j * 512 : (j + 1) * 512],
                start=True, stop=True,
            )
            nc.vector.tensor_scalar(
                dst[:, j * 512 : (j + 1) * 512], pp, b_sb, None, OP.add
            )

        kside = mk_side(NKT, "k")
        qside = mk_side(NQT, "q")
        kxn, qxn = kside[3], qside[3]

        # DVE order: k-chunks 0,1 -> q-chunk 0 (enough for the first scores),
        # then q-chunk 1, k-chunks 2,3 behind the first exps.
        stats_chunk(kx_sb, kside, 0, "k0")
        transpose_chunk(kxn, kxT, 0)
        proj(kxT, wk_sb, bk_sb, kT, 0)
        stats_chunk(kx_sb, kside, 1, "k1")
        transpose_chunk(kxn, kxT, 1)
        proj(kxT, wk_sb, bk_sb, kT, 1)
        stats_chunk(qx_sb, qside, 0, "q0")
        transpose_chunk(qxn, qxT, 0)
        proj(qxT, wq_sb, bq_sb, qT, 0)
        # rest of the stats run behind the first score/exp wave
        stats_chunk(qx_sb, qside, 1, "q1")
        transpose_chunk(qxn, qxT, 1)
        stats_chunk(kx_sb, kside, 2, "k2")
        transpose_chunk(kxn, kxT, 2)
        stats_chunk(kx_sb, kside, 3, "k3")
        transpose_chunk(kxn, kxT, 3)
        # (projections for qT j1 / kT j2,j3 are emitted inside the qc==0
        # block of the main loop, after the half0 scores, so the PE order
        # doesn't gate the first exps on the tail of the stats chain.)
        pre_ctx.close()

        # ---- software-pipelined main loop (lag-2) ----
        with (
            tc.tile_pool(name="spsum", bufs=3, space="PSUM") as spool,
            tc.tile_pool(name="opsum", bufs=1, space="PSUM") as opool,
            tc.tile_pool(name="pwork", bufs=2) as pwork,
            tc.tile_pool(name="awork", bufs=2) as awork,
            tc.tile_pool(name="owork", bufs=2) as owork,
            tc.tile_pool(name="small", bufs=2) as small,
        ):
            st = {}
            DVH = DV // 2  # 384: one PSUM bank per dv-half

            def emit_apath_part(qc, kh, w=1024):
                # A(qc) chunk kh (width w): sum_h P_h(qc)/R_h(qc); then xbar
                s = st[qc]
                P, crec = s["P"], s["crec"]
                if "A" not in s:
                    s["A"] = awork.tile([128, NK], f16, tag="A", name=f"A_{qc}")
                    s["AT"] = awork.tile([128, NK], f16, tag="AT", name=f"AT_{qc}")
                A, AT = s["A"], s["AT"]
                ksl = slice(kh * w, (kh + 1) * w)
                t1 = awork.tile(
                    [128, 1024], f16, tag="t1", name=f"t1_{qc}_{kh}_{w}"
                )[:, 0:w]
                t2 = awork.tile(
                    [128, 1024], f16, tag="t2", name=f"t2_{qc}_{kh}_{w}"
                )[:, 0:w]
                t3 = awork.tile(
                    [128, 1024], f16, tag="t3", name=f"t3_{qc}_{kh}_{w}"
                )[:, 0:w]
                nc.vector.tensor_scalar_mul(A[:, ksl], P[:, 0, ksl], crec[:, 0:1])
                nc.vector.tensor_scalar_mul(t1, P[:, 1, ksl], crec[:, 1:2])
                nc.vector.tensor_scalar_mul(t2, P[:, 2, ksl], crec[:, 2:3])
                nc.vector.tensor_scalar_mul(t3, P[:, 3, ksl], crec[:, 3:4])
                nc.vector.tensor_add(t2, t2, t3)
                nc.vector.tensor_add(t1, t1, A[:, ksl])
                nc.vector.tensor_add(A[:, ksl], t1, t2)
                nc.sync.dma_start_transpose(
                    out=AT[:, ksl].rearrange("p (c j) -> p c j", j=128),
                    in_=A[:, ksl],
                )

            def emit_pv(qc, dvh, kcs):
                s = st[qc]
                key = f"O{dvh}"
                if key not in s:
                    s[key] = opool.tile(
                        [128, DVH], fp32, tag=key, name=f"{key}_{qc}"
                    )
                O, AT = s[key], s["AT"]
                for kc in kcs:
                    nc.tensor.matmul(
                        O,
                        lhsT=AT[:, kc * 128 : (kc + 1) * 128],
                        rhs=v_sb[:, kc, dvh * DVH : (dvh + 1) * DVH],
                        start=kc == 0,
                        stop=kc == NKT - 1,
                    )

            def emit_evict(qc, dvh):
                s = st[qc]
                if "osb" not in s:
                    s["osb"] = owork.tile(
                        [128, DV], fp32, tag="osb", name=f"osb_{qc}"
                    )
                nc.vector.tensor_copy(
                    s["osb"][:, dvh * DVH : (dvh + 1) * DVH], s[f"O{dvh}"]
                )

            # q-block qc holds tokens {j*NQT + qc}: strided rows in o_d
            o_view = o_d.rearrange("(j c) d -> c j d", c=NQT)

            def emit_out(qc, dvh=None):
                if dvh is None:
                    nc.sync.dma_start(out=o_view[qc], in_=st[qc]["osb"])
                else:
                    sl = slice(dvh * DVH, (dvh + 1) * DVH)
                    nc.sync.dma_start(
                        out=o_view[qc][:, :, sl] if len(o_view[qc].shape) == 3
                        else o_view[qc][:, sl],
                        in_=st[qc]["osb"][:, sl],
                    )

            def emit_scores(qc, h, half):
                # S(qc, h, half): [128, 1024] psum, two 512 matmuls, then exp
                qsl = slice(qc * 128, (qc + 1) * 128)
                s = st[qc]
                S = spool.tile(
                    [128, 1024], fp32, tag="S", name=f"S_{qc}_{h}_{half}"
                )
                for kc in range(2):
                    ko = half * 1024 + kc * 512
                    nc.tensor.matmul(
                        S[:, kc * 512 : (kc + 1) * 512],
                        lhsT=qT[32 * h : 32 * (h + 1), qsl],
                        rhs=kT[32 * h : 32 * (h + 1), ko : ko + 512],
                        start=True,
                        stop=True,
                        tile_position=(32 * h, 0),
                    )
                nc.scalar.activation(
                    P_slice(s, h, half), S, AF.Exp,
                    accum_out=s["racc"][:, h, half : half + 1],
                )

            def P_slice(s, h, half):
                return s["P"][:, h, half * 1024 : (half + 1) * 1024]

            def mk_state(qc):
                P = pwork.tile([128, H, NK], f16, tag="P", name=f"P_{qc}")
                racc = small.tile([128, H, 2], fp32, tag="racc", name=f"racc_{qc}")
                st[qc] = {"P": P, "racc": racc}

            def emit_crec(qc):
                rsum = small.tile([128, H], fp32, tag="rsum", name=f"rs_{qc}")
                nc.vector.tensor_add(
                    rsum, st[qc]["racc"][:, :, 0], st[qc]["racc"][:, :, 1]
                )
                crec = small.tile([128, H], fp32, tag="crec", name=f"cr_{qc}")
                nc.vector.reciprocal(crec, rsum)
                st[qc]["crec"] = crec

            def emit_dmy(dvh, n, nm):
                dmy = opool.tile(
                    [128, DVH], fp32, tag=f"O{dvh}", name=f"dmy{dvh}_{nm}"
                )
                for _ in range(n):
                    nc.tensor.matmul(
                        dmy, lhsT=kT[:, 0:128], rhs=kT[:, 0:DVH],
                        start=True, stop=True,
                    )

            for qc in range(NQT + 1):
                cur = qc if qc < NQT else None
                if 0 <= qc - 1 < NQT - 1:
                    emit_apath_part(qc - 1, 0)
                    emit_apath_part(qc - 1, 1)
                if qc == 0:
                    # halves-major first tile: half0 scores/exps run while the
                    # q1/k2/k3 stats+projections finish on DVE/Sync/PE.
                    mk_state(0)
                    for h in range(H):
                        emit_scores(0, h, 0)
                    emit_dmy(0, 8, "w0")
                    proj(qxT, wq_sb, bq_sb, qT, 1, psum_pool=spool)
                    proj(kxT, wk_sb, bk_sb, kT, 2, psum_pool=spool)
                    proj(kxT, wk_sb, bk_sb, kT, 3, psum_pool=spool)
                    emit_dmy(1, 8, "w1")
                    for h in range(H):
                        emit_scores(0, h, 1)
                    emit_crec(0)
                elif cur is not None:
                    mk_state(qc)
                    for h in range(H):
                        # PV burst split around this h's score matmuls: a few
                        # MMs BEFORE them bridge the block-boundary stall, the
                        # rest come after so the scores reach ACT quickly.
                        if h in (0, 2):
                            dvh = h // 2
                            if qc - 2 >= 0:
                                emit_pv(qc - 2, dvh, range(0, 4))
                            else:
                                emit_dmy(dvh, 20, f"q{qc}h{h}")
                        elif qc >= 2:
                            # small keep-warm filler in the non-PV slots: keeps
                            # PE duty high so the utilization-driven HAM never
                            # re-throttles mid-kernel.
                            emit_dmy((h - 1) // 2, 3, f"q{qc}h{h}")
                        emit_scores(qc, h, 0)
                        emit_scores(qc, h, 1)
                        if h in (0, 2) and qc - 2 >= 0:
                            emit_pv(qc - 2, h // 2, range(4, NKT))
                            emit_evict(qc - 2, h // 2)
                            emit_out(qc - 2, h // 2)
                    emit_crec(qc)
                else:
                    # qc == NQT: drain the last two qtiles. A(NQT-1) combines
                    # in 512-wide quarters on DVE while PV(NQT-2) occupies the
                    # PE; PV(NQT-1) follows each transposed quarter.
                    last = NQT - 1
                    emit_apath_part(last, 0, w=512)
                    emit_apath_part(last, 1, w=512)
                    emit_pv(qc - 2, 0, range(NKT))
                    emit_evict(qc - 2, 0)
                    emit_out(qc - 2, 0)
                    emit_apath_part(last, 2, w=512)
                    emit_apath_part(last, 3, w=512)
                    emit_pv(qc - 2, 1, range(NKT))
                    emit_evict(qc - 2, 1)
                    emit_out(qc - 2, 1)
                    emit_pv(last, 0, range(0, 8))
                    emit_pv(last, 1, range(0, 8))
                    emit_pv(last, 0, range(8, NKT))
                    emit_evict(last, 0)
                    emit_out(last, 0)
                    emit_pv(last, 1, range(8, NKT))
                    emit_evict(last, 1)
                    emit_out(last, 1)


def _get_nc():
    if "nc" not in _CACHE:
        _CACHE["nc"] = _build_nc()
    return _CACHE["nc"]


def _host_prep(query, key, value, rms_q_w, rms_k_w, Wq, Wk, bq, bk):
    s = np.sqrt(float(HD))
    wqt = (rms_q_w[:, None] * Wq.T / s).astype(np.float16)
    wkt = (rms_k_w[:, None] * Wk.T).astype(np.float16)
    bqe = (bq / s).astype(np.float32)
    bke = bk.astype(np.float32)
    vq = (0.25 * value).astype(np.float16)  # [B, NK, DV]
    in_maps = []
    nq_blk = NQ // (N_CORES // B)  # 1024
    for c in range(N_CORES):
        b, qi = divmod(c, N_CORES // B)
        in_maps.append(
            {
                "q": np.ascontiguousarray(
                    query[b, qi * nq_blk : (qi + 1) * nq_blk]
                ).astype(np.float16),
                "k": np.ascontiguousarray(key[b]).astype(np.float16),
                "v": np.ascontiguousarray(vq[b]),
                "wqt": wqt,
                "wkt": wkt,
                "bqe": bqe,
                "bke": bke,
            }
        )
    return in_maps


def kernel(query, key, value, rms_q_w, rms_k_w, Wq, Wk, bq, bk, _trace=False):
    from concourse import bass_utils

    in_maps = _host_prep(
        np.asarray(query), np.asarray(key), np.asarray(value),
        np.asarray(rms_q_w), np.asarray(rms_k_w),
        np.asarray(Wq), np.asarray(Wk), np.asarray(bq), np.asarray(bk),
    )
    nc = _get_nc()
    res = bass_utils.run_bass_kernel_spmd(
        nc, in_maps, core_ids=list(range(N_CORES)), trace=_trace
    )
    _CACHE["last_results"] = res
    outs = [np.asarray(r["o"], dtype=np.float32) for r in res.results]
    nq_blk = NQ // (N_CORES // B)
    out = np.empty((B, NQ, DV), dtype=np.float32)
    for c in range(N_CORES):
        b, qi = divmod(c, N_CORES // B)
        out[b, qi * nq_blk : (qi + 1) * nq_blk] = outs[c]
    return out
